# revision 6
# baseline (speedup 1.0000x reference)
"""AELoss (associative-embedding push/pull loss) on 8 TRN2 NeuronCores.

Strategy: data-parallel over batch — each of the 8 cores handles 4 images.
The tags tensor is huge ([B, N, 1], N = 17*256*256) but only M*K = 510
elements per image are ever read, so the kernel gathers exactly those
elements with one indirect DMA (SWDGE gather) per core and does the whole
per-person / per-image reduction on-chip:

  partitions 0..119 = persons (4 images x 30 persons), free dim = 17 joints:
  per-person mean / pull via free-dim reduces; per-image sums via one tiny
  PE matmul against a constant selection matrix; the pairwise push term on a
  [4, 30, 30] broadcast layout (partition = image); output [4, 2] per core.

All small per-core inputs ride in ONE packed [128, 68] f32 DMA (vis, sel,
one-hot, idx-as-bits) to keep the DMA-lane semaphore count low — the
kernel-tail Drain instruction has a hard cap on sync waits.
"""

import numpy as np

B, M, K = 32, 30, 17
N = 17 * 256 * 256
NCORES = 8
BL = B // NCORES          # images per core
P = 128
PERS = BL * M             # persons per core (120)

# packed layout (f32 columns)
C_VIS = 0                 # [0, 17)   visibility
C_SEL = K                 # [17, 21)  sel[p, b] = person p belongs to image b
C_ONEH = K + BL           # [21, 51)  oneh[p, j] = (p % 30 == j)
C_IDX = K + BL + M        # [51, 68)  gather index, int32 bits
W_PACK = C_IDX + K        # 68

_cache = {}


def _constants():
    p = np.arange(P)
    m = p < PERS
    sel = np.zeros((P, BL), np.float32)
    sel[p[m], (p // M)[m]] = 1.0
    oneh = np.zeros((P, M), np.float32)
    oneh[p[m], (p % M)[m]] = 1.0
    return sel, oneh


def _build():
    import concourse.bass as bass
    import concourse.bacc as bacc
    import concourse.mybir as mybir
    from concourse.tile import TileContext

    f32 = mybir.dt.float32
    i32 = mybir.dt.int32
    X = mybir.AxisListType.X
    op = mybir.AluOpType

    nc = bacc.Bacc(trn_type="TRN2")
    tags_d = nc.dram_tensor("tags", [BL * N, 1], f32, kind="ExternalInput")
    packed_d = nc.dram_tensor("packed", [P, W_PACK], f32, kind="ExternalInput")
    out_d = nc.dram_tensor("out", [BL, 2], f32, kind="ExternalOutput")

    with TileContext(nc) as tc:
        with (
            tc.tile_pool(name="sb", bufs=1) as sb,
            tc.tile_pool(name="ps", bufs=1, space="PSUM") as ps,
        ):
            packed_t = sb.tile([P, W_PACK], f32)
            nc.sync.dma_start(out=packed_t[:], in_=packed_d[:])
            vis_t = packed_t[:, C_VIS:C_VIS + K]
            oneh_t = packed_t[:, C_ONEH:C_ONEH + M]
            idx_t = packed_t[:, C_IDX:C_IDX + K].bitcast(i32)

            # stage the matmul weight tile through a DVE copy: PE's LDWEIGHTS
            # can carry only one sync wait, so it must depend on the DVE sem
            sel_t = sb.tile([P, BL], f32)
            nc.vector.tensor_copy(out=sel_t[:], in_=packed_t[:, C_SEL:C_SEL + BL])

            # gather g[p, k] = tags_flat[idx[p, k]] — the HW indirect DMA
            # consumes ONE index per partition (gathering a contiguous row per
            # index), so issue one gather per joint column
            g = sb.tile([P, K], f32)
            for k in range(K):
                nc.gpsimd.indirect_dma_start(
                    out=g[:, k:k + 1],
                    out_offset=None,
                    in_=tags_d[:],
                    in_offset=bass.IndirectOffsetOnAxis(ap=idx_t[:, k:k + 1], axis=0),
                )

            # per-person stats over the joint axis (free dim)
            cnt = sb.tile([P, 1], f32)
            nc.vector.reduce_sum(out=cnt[:], in_=vis_t, axis=X)
            gv = sb.tile([P, K], f32)
            nc.vector.tensor_mul(out=gv[:], in0=g[:], in1=vis_t)
            s = sb.tile([P, 1], f32)
            nc.vector.reduce_sum(out=s[:], in_=gv[:], axis=X)
            sc = sb.tile([P, 1], f32)
            nc.vector.tensor_scalar_max(out=sc[:], in0=cnt[:], scalar1=1.0)
            rc = sb.tile([P, 1], f32)
            nc.vector.reciprocal(out=rc[:], in_=sc[:])
            mean = sb.tile([P, 1], f32)
            nc.vector.tensor_mul(out=mean[:], in0=s[:], in1=rc[:])

            dev = sb.tile([P, K], f32)
            nc.vector.tensor_tensor(
                out=dev[:], in0=g[:], in1=mean[:].to_broadcast([P, K]), op=op.subtract
            )
            dev2 = sb.tile([P, K], f32)
            nc.vector.tensor_mul(out=dev2[:], in0=dev[:], in1=dev[:])
            dev2v = sb.tile([P, K], f32)
            nc.vector.tensor_mul(out=dev2v[:], in0=dev2[:], in1=vis_t)
            pp = sb.tile([P, 1], f32)
            nc.vector.reduce_sum(out=pp[:], in_=dev2v[:], axis=X)
            pullpp = sb.tile([P, 1], f32)
            nc.vector.tensor_mul(out=pullpp[:], in0=pp[:], in1=rc[:])

            valid = sb.tile([P, 1], f32)
            nc.vector.tensor_scalar(
                out=valid[:], in0=cnt[:], scalar1=0.5, scalar2=None, op0=op.is_gt
            )
            pvpp = sb.tile([P, 1], f32)
            nc.vector.tensor_mul(out=pvpp[:], in0=pullpp[:], in1=valid[:])

            # image-level sums via matmul: out[b, :] = sum_p sel[p, b] * rhs1[p, :]
            # cols 0:30  = mean scattered to person slot (mean * onehot)
            # cols 30:60 = valid scattered to person slot
            # col 60 = valid*pull_pp, col 61 = valid
            rhs1 = sb.tile([P, 2 * M + 2], f32)
            nc.vector.tensor_tensor(
                out=rhs1[:, 0:M], in0=oneh_t,
                in1=mean[:].to_broadcast([P, M]), op=op.mult,
            )
            nc.vector.tensor_tensor(
                out=rhs1[:, M:2 * M], in0=oneh_t,
                in1=valid[:].to_broadcast([P, M]), op=op.mult,
            )
            nc.vector.tensor_copy(out=rhs1[:, 2 * M:2 * M + 1], in_=pvpp[:])
            nc.vector.tensor_copy(out=rhs1[:, 2 * M + 1:2 * M + 2], in_=valid[:])

            m1 = ps.tile([BL, 2 * M + 2], f32)
            nc.tensor.matmul(out=m1[:], lhsT=sel_t[:], rhs=rhs1[:], start=True, stop=True)
            p1s = sb.tile([BL, 2 * M + 2], f32)
            nc.vector.tensor_copy(out=p1s[:], in_=m1[:])

            # pairwise push term on [image, i, j]: broadcast APs over M2/V2 rows
            m2v = p1s[:, 0:M].rearrange("p (m o) -> p m o", o=1)          # [4, 30, 1]
            v2v = p1s[:, M:2 * M].rearrange("p (m o) -> p m o", o=1)
            d4 = sb.tile([BL, M * M], f32)
            d4v = d4[:].rearrange("p (i j) -> p i j", i=M)
            nc.vector.tensor_tensor(
                out=d4v, in0=m2v.broadcast_to([BL, M, M]),
                in1=m2v.rearrange("p m o -> p o m").broadcast_to([BL, M, M]),
                op=op.subtract,
            )
            dd4 = sb.tile([BL, M * M], f32)
            nc.vector.tensor_mul(out=dd4[:], in0=d4[:], in1=d4[:])
            e4 = sb.tile([BL, M * M], f32)
            nc.scalar.activation(
                out=e4[:], in_=dd4[:],
                func=mybir.ActivationFunctionType.Exp, bias=0.0, scale=-1.0,
            )
            pm4 = sb.tile([BL, M * M], f32)
            nc.vector.tensor_tensor(
                out=pm4[:].rearrange("p (i j) -> p i j", i=M),
                in0=v2v.broadcast_to([BL, M, M]),
                in1=v2v.rearrange("p m o -> p o m").broadcast_to([BL, M, M]),
                op=op.mult,
            )
            em4 = sb.tile([BL, M * M], f32)
            nc.vector.tensor_mul(out=em4[:], in0=e4[:], in1=pm4[:])
            S = sb.tile([BL, 1], f32)
            nc.vector.reduce_sum(out=S[:], in_=em4[:], axis=X)

            # final per-image scalars on partitions 0..3
            nn = p1s[:, 2 * M + 1:2 * M + 2]
            pn = p1s[:, 2 * M:2 * M + 1]
            sn = sb.tile([BL, 1], f32)
            nc.vector.tensor_scalar_max(out=sn[:], in0=nn, scalar1=1.0)
            r1 = sb.tile([BL, 1], f32)
            nc.vector.reciprocal(out=r1[:], in_=sn[:])
            gt0 = sb.tile([BL, 1], f32)
            nc.vector.tensor_scalar(
                out=gt0[:], in0=nn, scalar1=0.5, scalar2=None, op0=op.is_gt
            )
            pull0 = sb.tile([BL, 1], f32)
            nc.vector.tensor_mul(out=pull0[:], in0=pn, in1=r1[:])
            pull = sb.tile([BL, 1], f32)
            nc.vector.tensor_mul(out=pull[:], in0=pull0[:], in1=gt0[:])

            ge2 = sb.tile([BL, 1], f32)
            nc.vector.tensor_scalar(
                out=ge2[:], in0=nn, scalar1=1.5, scalar2=None, op0=op.is_gt
            )
            nm1 = sb.tile([BL, 1], f32)
            nc.vector.tensor_scalar_add(out=nm1[:], in0=nn, scalar1=-1.0)
            den = sb.tile([BL, 1], f32)
            nc.vector.tensor_mul(out=den[:], in0=nn, in1=nm1[:])
            dens = sb.tile([BL, 1], f32)
            nc.vector.tensor_scalar_max(out=dens[:], in0=den[:], scalar1=1.0)
            r2 = sb.tile([BL, 1], f32)
            nc.vector.reciprocal(out=r2[:], in_=dens[:])
            smn = sb.tile([BL, 1], f32)
            nc.vector.tensor_sub(out=smn[:], in0=S[:], in1=nn)
            push0 = sb.tile([BL, 1], f32)
            nc.vector.tensor_mul(out=push0[:], in0=smn[:], in1=r2[:])
            push1 = sb.tile([BL, 1], f32)
            nc.vector.tensor_scalar_mul(out=push1[:], in0=push0[:], scalar1=0.5)
            push = sb.tile([BL, 1], f32)
            nc.vector.tensor_mul(out=push[:], in0=push1[:], in1=ge2[:])

            outt = sb.tile([BL, 2], f32)
            nc.vector.tensor_copy(out=outt[:, 0:1], in_=push[:])
            nc.vector.tensor_copy(out=outt[:, 1:2], in_=pull[:])
            nc.sync.dma_start(out=out_d[:], in_=outt[:])

    nc.compile()
    return nc


def _in_maps(tags, joints):
    sel, oneh = _constants()
    tags = np.ascontiguousarray(np.asarray(tags, dtype=np.float32)).reshape(B, N)
    joints = np.asarray(joints, dtype=np.int32)
    idx_all = joints[..., 0]                               # [B, M, K]
    vis_all = (joints[..., 1] > 0).astype(np.float32)      # [B, M, K]

    in_maps = []
    for c in range(NCORES):
        b0 = c * BL
        packed = np.zeros((P, W_PACK), np.float32)
        idx_l = np.zeros((P, K), np.int32)
        for b in range(BL):
            rows = slice(b * M, (b + 1) * M)
            idx_l[rows] = idx_all[b0 + b] + b * N
            packed[rows, C_VIS:C_VIS + K] = vis_all[b0 + b]
        packed[:, C_SEL:C_SEL + BL] = sel
        packed[:, C_ONEH:C_ONEH + M] = oneh
        packed[:, C_IDX:C_IDX + K] = idx_l.view(np.float32)
        in_maps.append({
            "tags": np.ascontiguousarray(tags[b0:b0 + BL].reshape(BL * N, 1)),
            "packed": packed,
        })
    return in_maps


def _run(in_maps, trace=False):
    from concourse import bass_utils

    if "nc" not in _cache:
        _cache["nc"] = _build()
    return bass_utils.run_bass_kernel_spmd(
        _cache["nc"], in_maps, core_ids=list(range(NCORES)), trace=trace
    )


def kernel(tags, joints):
    res = _run(_in_maps(tags, joints))
    outs = [res.results[c]["out"] for c in range(NCORES)]
    push = np.concatenate([o[:, 0] for o in outs]).astype(np.float32)
    pull = np.concatenate([o[:, 1] for o in outs]).astype(np.float32)
    return push, pull
